# revision 4
# baseline (speedup 1.0000x reference)
"""Trainium2 Bass kernel for nn_MetaphorModel (masked segment-mean pool +
tiny linear classifier + CE loss).

Strategy (pure data parallel, 8 NeuronCores):
  - Shard batch B=256 across 8 cores (32 samples/core).
  - Per core, the masked mean-pool over S is a matmul: for each 128-row
    chunk of hidden states, lhsT is a [128, 32] "indicator" matrix whose
    column b holds mask[b, s]/count[b] for the rows of sample b in that
    chunk (0 elsewhere). PSUM accumulates pooled [32, 768] across chunks.
  - Classifier (768 -> 2) runs on the vector engine as two fused
    multiply+reduce ops against the replicated weight rows, bias folded
    into the reduction's initial value. Logits [32, 2] DMA'd out.
  - Host computes the scalar CE loss from the gathered [256, 2] logits
    (trivial epilogue) and returns (loss, logits) like the reference.
"""

import numpy as np

_B, _S, _D, _C = 256, 512, 768, 2
_NCORES = 8
_BPC = _B // _NCORES        # samples per core = 32
_P = 128                    # SBUF partitions
_SCH = _S // _P             # 128-row chunks per sample = 4
_NCH = _BPC * _SCH          # chunks per core = 128

# Stash of the last run's BassKernelResults (exec_time_ns etc.) for test
# harness introspection; not used for grading.
LAST_RESULTS = None


def _build_program():
    """Build + compile the per-core Bass/Tile program. Returns nc."""
    from contextlib import ExitStack

    import concourse.bacc as bacc
    import concourse.mybir as mybir
    import concourse.tile as tile

    f32 = mybir.dt.float32
    nc = bacc.Bacc("TRN2", target_bir_lowering=False, debug=False,
                   num_devices=_NCORES)

    h_d = nc.dram_tensor("h", [_BPC, _S, _D], f32, kind="ExternalInput")
    ind_d = nc.dram_tensor("ind", [_P, _NCH * _BPC], f32, kind="ExternalInput")
    w_d = nc.dram_tensor("wrep", [_BPC, _C * _D], f32, kind="ExternalInput")
    b_d = nc.dram_tensor("brep", [_BPC, _C], f32, kind="ExternalInput")
    lg_d = nc.dram_tensor("logits", [_BPC, _C], f32, kind="ExternalOutput")

    with tile.TileContext(nc) as tc, ExitStack() as ctx:
        hpool = ctx.enter_context(tc.tile_pool(name="h", bufs=6))
        cpool = ctx.enter_context(tc.tile_pool(name="const", bufs=1))
        ppool = ctx.enter_context(tc.tile_pool(name="ps", bufs=1, space="PSUM"))
        epool = ctx.enter_context(tc.tile_pool(name="ep", bufs=1))

        ind_sb = cpool.tile([_P, _NCH * _BPC], f32)
        nc.sync.dma_start(ind_sb[:], ind_d.ap())
        wrep = cpool.tile([_BPC, _C * _D], f32)
        nc.sync.dma_start(wrep[:], w_d.ap())
        brep = cpool.tile([_BPC, _C], f32)
        nc.sync.dma_start(brep[:], b_d.ap())

        pooled = ppool.tile([_BPC, _D], f32)

        for g in range(_BPC):
            ht = hpool.tile([_P, _SCH, _D], f32)
            src = h_d.ap()[g].rearrange("(c p) d -> p c d", p=_P)
            nc.sync.dma_start(ht[:], src)
            for c in range(_SCH):
                k = g * _SCH + c
                lhsT = ind_sb[:, k * _BPC:(k + 1) * _BPC]
                first, last = (k == 0), (k == _NCH - 1)
                nc.tensor.matmul(pooled[:, 0:512], lhsT, ht[:, c, 0:512],
                                 start=first, stop=last)
                nc.tensor.matmul(pooled[:, 512:_D], lhsT, ht[:, c, 512:_D],
                                 start=first, stop=last)

        lg = epool.tile([_BPC, _C], f32)
        for ci in range(_C):
            prod = epool.tile([_BPC, _D], f32, tag=f"prod{ci}")
            nc.vector.tensor_mul(prod[:], pooled[:],
                                 wrep[:, ci * _D:(ci + 1) * _D])
            nc.vector.reduce_sum(lg[:, ci:ci + 1], prod[:],
                                 axis=mybir.AxisListType.X)
        nc.vector.tensor_add(lg[:], lg[:], brep[:])
        nc.sync.dma_start(lg_d.ap(), lg[:])

    nc.compile()
    return nc


def _prep_in_maps(h, mask, W, bias):
    maskf = mask.astype(np.float32)
    counts = maskf.sum(axis=1)                      # [B]
    maskw = maskf / counts[:, None]                 # [B, S]

    wrep_np = np.ascontiguousarray(
        np.broadcast_to(W.reshape(1, _C * _D), (_BPC, _C * _D)))
    brep_np = np.ascontiguousarray(
        np.broadcast_to(bias.reshape(1, _C), (_BPC, _C)))

    in_maps = []
    for core in range(_NCORES):
        sl = slice(core * _BPC, (core + 1) * _BPC)
        hc = np.ascontiguousarray(h[sl])            # [32, 512, 768]
        mw = maskw[sl]                              # [32, 512]
        # indicator weights: chunk k = (g, c) covers rows c*128..c*128+127
        # of sample g; column g gets mask/count, all others zero.
        ind = np.zeros((_P, _NCH * _BPC), dtype=np.float32)
        for g in range(_BPC):
            for c in range(_SCH):
                k = g * _SCH + c
                ind[:, k * _BPC + g] = mw[g, c * _P:(c + 1) * _P]
        in_maps.append({"h": hc, "ind": ind, "wrep": wrep_np,
                        "brep": brep_np})
    return in_maps


def kernel(last_hidden_state, metaphor_mask, labels, classifier_w,
           classifier_b):
    global LAST_RESULTS
    from concourse.bass_utils import run_bass_kernel_spmd

    h = np.asarray(last_hidden_state, dtype=np.float32)
    mask = np.asarray(metaphor_mask)
    labels = np.asarray(labels)
    W = np.asarray(classifier_w, dtype=np.float32)
    bias = np.asarray(classifier_b, dtype=np.float32)

    assert h.shape == (_B, _S, _D) and W.shape == (_C, _D)

    nc = _build_program()
    in_maps = _prep_in_maps(h, mask, W, bias)

    res = run_bass_kernel_spmd(nc, in_maps, core_ids=list(range(_NCORES)))
    LAST_RESULTS = res
    logits = np.concatenate([r["logits"] for r in res.results], axis=0)

    # Host epilogue: CE loss (mean reduction) over the tiny [256, 2] logits.
    lg64 = logits.astype(np.float64)
    m = lg64.max(axis=1, keepdims=True)
    lse = (m[:, 0] + np.log(np.exp(lg64 - m).sum(axis=1)))
    nll = lse - lg64[np.arange(_B), labels.astype(np.int64)]
    loss = np.float32(nll.mean())
    return loss, logits


def benchmark(np_inputs, iters=20):
    """Dev-only timing helper (not used for grading): builds the PJRT
    executable once, pre-places inputs on the 8 cores, and times repeated
    executions. Returns (min_ns, all_ns)."""
    import time

    import jax
    import numpy as jnp_np
    from jax.sharding import Mesh, PartitionSpec
    from jax.experimental.shard_map import shard_map

    import concourse.mybir as mybir
    from concourse.bass2jax import _bass_exec_p, install_neuronx_cc_hook

    h = np.asarray(np_inputs["last_hidden_state"], dtype=np.float32)
    mask = np.asarray(np_inputs["metaphor_mask"])
    W = np.asarray(np_inputs["classifier_w"], dtype=np.float32)
    bias = np.asarray(np_inputs["classifier_b"], dtype=np.float32)

    nc = _build_program()
    in_maps = _prep_in_maps(h, mask, W, bias)

    install_neuronx_cc_hook()
    in_names, out_names, out_avals, zero_outs = [], [], [], []
    for alloc in nc.m.functions[0].allocations:
        if not isinstance(alloc, mybir.MemoryLocationSet):
            continue
        name = alloc.memorylocations[0].name
        if alloc.kind == "ExternalInput":
            in_names.append(name)
        elif alloc.kind == "ExternalOutput":
            shape = tuple(alloc.tensor_shape)
            dtype = mybir.dt.np(alloc.dtype)
            out_names.append(name)
            out_avals.append(jax.core.ShapedArray(shape, dtype))
            zero_outs.append(np.zeros(shape, dtype))
    n_params = len(in_names)
    all_names = in_names + out_names

    def _body(*args):
        outs = _bass_exec_p.bind(
            *args,
            out_avals=tuple(out_avals),
            in_names=tuple(all_names),
            out_names=tuple(out_names),
            lowering_input_output_aliases=(),
            sim_require_finite=True,
            sim_require_nnan=True,
            nc=nc,
        )
        return tuple(outs)

    devices = jax.devices()[:_NCORES]
    mesh = Mesh(np.asarray(devices), ("core",))
    n_outs = len(out_names)
    in_specs = (PartitionSpec("core"),) * (n_params + n_outs)
    out_specs = (PartitionSpec("core"),) * n_outs
    donate = tuple(range(n_params, n_params + n_outs))
    sharded = jax.jit(
        shard_map(_body, mesh=mesh, in_specs=in_specs, out_specs=out_specs,
                  check_rep=False),
        donate_argnums=donate, keep_unused=True)

    concat_in = [
        np.concatenate([in_maps[c][name] for c in range(_NCORES)], axis=0)
        for name in in_names
    ]
    # commit inputs to devices once so timing excludes host->device copies
    from jax.sharding import NamedSharding
    sharding = NamedSharding(mesh, PartitionSpec("core"))
    dev_in = [jax.device_put(x, sharding) for x in concat_in]

    def fresh_zeros():
        return [jax.device_put(
            np.zeros((_NCORES * z.shape[0], *z.shape[1:]), z.dtype), sharding)
            for z in zero_outs]

    # warmup (compiles)
    out = sharded(*dev_in, *fresh_zeros())
    jax.block_until_ready(out)

    times = []
    for _ in range(iters):
        zs = fresh_zeros()
        jax.block_until_ready(zs)
        t0 = time.perf_counter()
        out = sharded(*dev_in, *zs)
        jax.block_until_ready(out)
        times.append((time.perf_counter() - t0) * 1e9)
    logits = np.concatenate(
        [np.asarray(out[out_names.index("logits")])], axis=0)
    return min(times), times, logits


# revision 6
# speedup vs baseline: 1.4609x; 1.4609x over previous
"""Trainium2 Bass kernel for nn_MetaphorModel (masked segment-mean pool +
tiny linear classifier + CE loss).

Strategy (pure data parallel, 8 NeuronCores):
  - Shard batch B=256 across 8 cores (32 samples/core).
  - Only ~half the S=512 token rows are masked-in; the device gathers
    just those rows from HBM with indirect (gather) DMA, cutting HBM
    traffic ~2x vs a dense read. Each indirect DMA moves 128 runs of
    consecutive masked rows (one run per SBUF partition): "pair" DMAs
    carry 2-row runs (6 KB/partition), "single" DMAs carry 1-row runs.
    Run decomposition of the mask happens on host from the tiny
    [256, 512] bool mask; the 384 MB of hidden states only ever moves
    device-side.
  - The masked mean-pool is a matmul over the packed rows: for each
    gathered 128-row chunk, lhsT is a [128, 32] "indicator" matrix whose
    column b holds 1/count[b] at rows belonging to sample b (0
    elsewhere, 0 for pad rows). PSUM accumulates pooled [32, 768]
    across all chunks.
  - Classifier (768 -> 2) runs on the vector engine as two
    multiply+reduce ops against replicated weight rows, plus bias.
    Logits [32, 2] DMA'd out per core.
  - Host computes the scalar CE loss from the gathered [256, 2] logits
    (trivial epilogue) and returns (loss, logits) like the reference.
"""

import numpy as np

_B, _S, _D, _C = 256, 512, 768, 2
_NCORES = 8
_BPC = _B // _NCORES        # samples per core = 32
_P = 128                    # SBUF partitions

# Stash of the last run's BassKernelResults for dev introspection.
LAST_RESULTS = None


def _build_program(n_pair, n_single):
    """Build + compile the per-core Bass/Tile program for `n_pair`
    pair-run gathers and `n_single` single-run gathers. Returns nc."""
    from contextlib import ExitStack

    import concourse.bacc as bacc
    import concourse.bass as bass
    import concourse.mybir as mybir
    import concourse.tile as tile

    f32 = mybir.dt.float32
    i32 = mybir.dt.int32
    nch = 2 * n_pair + n_single
    nc = bacc.Bacc("TRN2", target_bir_lowering=False, debug=False,
                   num_devices=_NCORES)

    h_d = nc.dram_tensor("h", [_BPC * _S, _D], f32, kind="ExternalInput")
    ixp_d = nc.dram_tensor("ixp", [_P, max(n_pair, 1)], i32,
                           kind="ExternalInput")
    ixs_d = nc.dram_tensor("ixs", [_P, max(n_single, 1)], i32,
                           kind="ExternalInput")
    ind_d = nc.dram_tensor("ind", [_P, nch * _BPC], f32, kind="ExternalInput")
    w_d = nc.dram_tensor("wrep", [_BPC, _C * _D], f32, kind="ExternalInput")
    b_d = nc.dram_tensor("brep", [_BPC, _C], f32, kind="ExternalInput")
    lg_d = nc.dram_tensor("logits", [_BPC, _C], f32, kind="ExternalOutput")

    with tile.TileContext(nc) as tc, ExitStack() as ctx:
        ppool = ctx.enter_context(tc.tile_pool(name="hp", bufs=6))
        spool = ctx.enter_context(tc.tile_pool(name="hs", bufs=6))
        cpool = ctx.enter_context(tc.tile_pool(name="const", bufs=1))
        pspool = ctx.enter_context(tc.tile_pool(name="ps", bufs=1,
                                                space="PSUM"))
        epool = ctx.enter_context(tc.tile_pool(name="ep", bufs=1))

        ixp_sb = cpool.tile([_P, max(n_pair, 1)], i32)
        nc.sync.dma_start(ixp_sb[:], ixp_d.ap())
        ixs_sb = cpool.tile([_P, max(n_single, 1)], i32)
        nc.sync.dma_start(ixs_sb[:], ixs_d.ap())
        ind_sb = cpool.tile([_P, nch * _BPC], f32)
        nc.sync.dma_start(ind_sb[:], ind_d.ap())
        wrep = cpool.tile([_BPC, _C * _D], f32)
        nc.sync.dma_start(wrep[:], w_d.ap())
        brep = cpool.tile([_BPC, _C], f32)
        nc.sync.dma_start(brep[:], b_d.ap())

        pooled = pspool.tile([_BPC, _D], f32)
        k = 0

        def mm_chunk(rhs_full):
            nonlocal k
            lhsT = ind_sb[:, k * _BPC:(k + 1) * _BPC]
            first, last = (k == 0), (k == nch - 1)
            nc.tensor.matmul(pooled[:, 0:512], lhsT, rhs_full[:, 0:512],
                             start=first, stop=last)
            nc.tensor.matmul(pooled[:, 512:_D], lhsT, rhs_full[:, 512:_D],
                             start=first, stop=last)
            k += 1

        for g in range(n_pair):
            ht = ppool.tile([_P, 2 * _D], f32)
            nc.gpsimd.indirect_dma_start(
                out=ht[:], out_offset=None, in_=h_d.ap()[:],
                in_offset=bass.IndirectOffsetOnAxis(
                    ap=ixp_sb[:, g:g + 1], axis=0))
            mm_chunk(ht[:, 0:_D])
            mm_chunk(ht[:, _D:2 * _D])

        for j in range(n_single):
            ht = spool.tile([_P, _D], f32)
            nc.gpsimd.indirect_dma_start(
                out=ht[:], out_offset=None, in_=h_d.ap()[:],
                in_offset=bass.IndirectOffsetOnAxis(
                    ap=ixs_sb[:, j:j + 1], axis=0))
            mm_chunk(ht[:])

        lg = epool.tile([_BPC, _C], f32)
        for ci in range(_C):
            prod = epool.tile([_BPC, _D], f32, tag=f"prod{ci}")
            nc.vector.tensor_mul(prod[:], pooled[:],
                                 wrep[:, ci * _D:(ci + 1) * _D])
            nc.vector.reduce_sum(lg[:, ci:ci + 1], prod[:],
                                 axis=mybir.AxisListType.X)
        nc.vector.tensor_add(lg[:], lg[:], brep[:])
        nc.sync.dma_start(lg_d.ap(), lg[:])

    nc.compile()
    return nc


def _pack_runs(mask):
    """Decompose each sample's masked positions into pair runs (2
    consecutive masked s) and singles, greedily within maximal runs.

    Returns (pair_rows, single_rows, owners_pair, owners_single) per
    core: lists of flat row indices into the core's [BPC*S, D] shard."""
    m = mask  # [B, S] bool
    c = np.zeros_like(m, dtype=np.int32)  # position within run (1-based)
    c[:, 0] = m[:, 0]
    for s in range(1, _S):
        c[:, s] = (c[:, s - 1] + 1) * m[:, s]
    nxt = np.zeros_like(m)
    nxt[:, :-1] = m[:, 1:]
    pair_first = m & nxt & (c % 2 == 1)
    prev_pair_first = np.zeros_like(m)
    prev_pair_first[:, 1:] = pair_first[:, :-1]
    single = m & ~pair_first & ~prev_pair_first
    return pair_first, single


def _prep(h, mask, W, bias):
    """Returns (n_pair, n_single, in_maps)."""
    maskf = mask.astype(np.float32)
    counts = maskf.sum(axis=1)                      # [B]
    pair_first, single = _pack_runs(mask)

    per_core = []
    for core in range(_NCORES):
        sl = slice(core * _BPC, (core + 1) * _BPC)
        pb, ps = np.nonzero(pair_first[sl])
        sb, ss = np.nonzero(single[sl])
        per_core.append((pb, ps, sb, ss))

    n_pair = max((len(pb) + _P - 1) // _P for pb, _, _, _ in per_core)
    n_single = max((len(sb) + _P - 1) // _P for _, _, sb, _ in per_core)
    n_pair = max(n_pair, 1)
    n_single = max(n_single, 1)
    nch = 2 * n_pair + n_single

    wrep_np = np.ascontiguousarray(
        np.broadcast_to(W.reshape(1, _C * _D), (_BPC, _C * _D)))
    brep_np = np.ascontiguousarray(
        np.broadcast_to(bias.reshape(1, _C), (_BPC, _C)))

    in_maps = []
    for core in range(_NCORES):
        sl = slice(core * _BPC, (core + 1) * _BPC)
        hc = np.ascontiguousarray(h[sl].reshape(_BPC * _S, _D))
        cts = counts[sl]
        pb, ps, sb, ss = per_core[core]

        def pad_units(bs, ssv, n_units):
            rows = (bs * _S + ssv).astype(np.int32)
            w = (1.0 / cts[bs]).astype(np.float32)
            owner = bs.astype(np.int64)
            padn = n_units * _P - len(rows)
            rows = np.concatenate([rows, np.zeros(padn, np.int32)])
            w = np.concatenate([w, np.zeros(padn, np.float32)])
            owner = np.concatenate([owner, np.full(padn, -1)])
            # unit u -> DMA g = u // 128, partition p = u % 128
            return (rows.reshape(n_units, _P).T.copy(),
                    w.reshape(n_units, _P).T,
                    owner.reshape(n_units, _P).T)

        ixp, wp, op = pad_units(pb, ps, n_pair)
        ixs, ws, osg = pad_units(sb, ss, n_single)

        ind_np = np.zeros((_P, nch * _BPC), np.float32)
        pgrid = np.arange(_P)[:, None]
        # pair DMA g -> chunks 2g (first row) and 2g+1 (second row)
        for g in range(n_pair):
            v = op[:, g] >= 0
            ind_np[pgrid[v, 0], (2 * g) * _BPC + op[v, g]] = wp[v, g]
            ind_np[pgrid[v, 0], (2 * g + 1) * _BPC + op[v, g]] = wp[v, g]
        # single DMA j -> chunk 2*n_pair + j
        for j in range(n_single):
            v = osg[:, j] >= 0
            ind_np[pgrid[v, 0], (2 * n_pair + j) * _BPC + osg[v, j]] = \
                ws[v, j]

        in_maps.append({"h": hc, "ixp": np.ascontiguousarray(ixp),
                        "ixs": np.ascontiguousarray(ixs), "ind": ind_np,
                        "wrep": wrep_np, "brep": brep_np})
    return n_pair, n_single, in_maps


def kernel(last_hidden_state, metaphor_mask, labels, classifier_w,
           classifier_b):
    global LAST_RESULTS
    from concourse.bass_utils import run_bass_kernel_spmd

    h = np.asarray(last_hidden_state, dtype=np.float32)
    mask = np.asarray(metaphor_mask).astype(bool)
    labels = np.asarray(labels)
    W = np.asarray(classifier_w, dtype=np.float32)
    bias = np.asarray(classifier_b, dtype=np.float32)

    assert h.shape == (_B, _S, _D) and W.shape == (_C, _D)

    n_pair, n_single, in_maps = _prep(h, mask, W, bias)
    nc = _build_program(n_pair, n_single)

    res = run_bass_kernel_spmd(nc, in_maps, core_ids=list(range(_NCORES)))
    LAST_RESULTS = res
    logits = np.concatenate([r["logits"] for r in res.results], axis=0)

    # Host epilogue: CE loss (mean reduction) over the tiny [256, 2] logits.
    lg64 = logits.astype(np.float64)
    m = lg64.max(axis=1, keepdims=True)
    lse = (m[:, 0] + np.log(np.exp(lg64 - m).sum(axis=1)))
    nll = lse - lg64[np.arange(_B), labels.astype(np.int64)]
    loss = np.float32(nll.mean())
    return loss, logits
